# revision 9
# baseline (speedup 1.0000x reference)
"""CosineSimCodebook (VQ) forward + EMA update on 8 Trainium2 NeuronCores.

Strategy:
  Phase 1 (token-parallel, 2048 tokens/core): dist = x @ embed.T via f32r
    matmuls (full PE rate); argmax via DVE Max8/MaxIndex with an exact fp32
    top-2 rescoring pass (indirect-DMA gather of the two candidate embed rows
    + DVE dot products) to remove f32r rounding flips; quantize = selected
    gathered row.
  Phase 1.5: AllGather of the 16384 int32 indices (8 KB/rank).
  Phase 2 (code-parallel, 512 codes/core): rebuild one-hot masks from the
    gathered indices (iota+compare), accumulate embed_sum and bins over all
    16384 tokens with bf16 matmuls (ones-column trick folds bins into the
    same matmuls).
  Phase 3: EMA update + row l2norm on the code shard. new_embed reduces to
    l2norm(new_embed_avg) exactly, because the laplace-smoothed divisor is a
    positive per-row scalar that cancels under normalization.

Host side only shards/transpose-prepares inputs and concatenates outputs.
"""
import os
import sys
import numpy as np

sys.path.insert(0, "/opt/trn_rl_repo")

import ml_dtypes

from concourse import bacc, mybir
import concourse.bass as bass
import concourse.tile as tile
from concourse.bass_utils import run_bass_kernel_spmd

dt = mybir.dt
Alu = mybir.AluOpType

# problem shape (hardcoded per harness contract)
B, N, D, H, C = 16, 1024, 512, 1, 4096
NT = B * N                # 16384 tokens
NCORES = 8
NL = NT // NCORES         # 2048 tokens per core
NBLK = NL // 128          # 16 token blocks per core
CL = C // NCORES          # 512 codes per core
CCH = CL // 128           # 4 code chunks per core
NJ = NT // 128            # 128 global token chunks
DECAY = 0.8

_CACHE = {}


def _build():
    KPHASE = int(os.environ.get("KPHASE", "3"))
    nc = bacc.Bacc("TRN2", target_bir_lowering=False, debug=False,
                   num_devices=NCORES)

    # ---- I/O ----
    xT = nc.dram_tensor("xT", [D, NL], dt.float32r, kind="ExternalInput").ap()
    xnat = nc.dram_tensor("xnat", [NL, D], dt.float32, kind="ExternalInput").ap()
    embT = nc.dram_tensor("embT", [D, C], dt.float32r, kind="ExternalInput").ap()
    emb = nc.dram_tensor("emb", [C, D], dt.float32, kind="ExternalInput").ap()
    xaug = nc.dram_tensor("xaug", [NT, 514], dt.bfloat16, kind="ExternalInput").ap()
    ea_sh = nc.dram_tensor("ea_sh", [CL, D], dt.float32, kind="ExternalInput").ap()
    cs_sh = nc.dram_tensor("cs_sh", [CL], dt.float32, kind="ExternalInput").ap()
    cbase = nc.dram_tensor("cbase", [128, 1], dt.float32, kind="ExternalInput").ap()

    dist_o = nc.dram_tensor("dist_o", [NL, C], dt.float32, kind="ExternalOutput").ap()
    ind_o = nc.dram_tensor("ind_o", [NL], dt.int32, kind="ExternalOutput").ap()
    quant_o = nc.dram_tensor("quant_o", [NL, D], dt.float32, kind="ExternalOutput").ap()
    nemb_o = nc.dram_tensor("nemb_o", [CL, D], dt.float32, kind="ExternalOutput").ap()
    nea_o = nc.dram_tensor("nea_o", [CL, D], dt.float32, kind="ExternalOutput").ap()
    ncs_o = nc.dram_tensor("ncs_o", [CL], dt.float32, kind="ExternalOutput").ap()

    with tile.TileContext(nc) as tc:
        with (
            tc.tile_pool(name="persist", bufs=1) as pp,
            tc.tile_pool(name="dram", bufs=1, space="DRAM") as dr,
        ):
            iota_t = pp.tile([128, 512], dt.int16)
            nc.gpsimd.iota(iota_t[:], pattern=[[1, 512]], base=0,
                           channel_multiplier=0)
            cbase_t = pp.tile([128, 1], dt.float32)
            nc.sync.dma_start(cbase_t[:], cbase[:])
            # corrected local indices, block-per-column layouts
            indf_loc = pp.tile([128, NBLK], dt.float32)
            indi_loc = pp.tile([128, NBLK], dt.int32)

            # ================= Phase 1 =================
            with (
                tc.tile_pool(name="p1", bufs=1) as p1,
                tc.tile_pool(name="distp", bufs=2) as distp,
                tc.tile_pool(name="blk", bufs=2) as blk,
                tc.tile_pool(name="ps1", bufs=6, space="PSUM") as ps1,
            ):
                xT_t = p1.tile([128, 4, NL], dt.float32r)
                xnat_t = p1.tile([128, NBLK, D], dt.float32)
                embT_t = p1.tile([128, 4, C], dt.float32r)
                nc.sync.dma_start(xT_t[:], xT.rearrange("(k p) n -> p k n", p=128))
                nc.sync.dma_start(xnat_t[:], xnat.rearrange("(j p) d -> p j d", p=128))
                nc.sync.dma_start(embT_t[:], embT.rearrange("(k p) c -> p k c", p=128))

                for i in range(NBLK):
                    dist_t = distp.tile([128, C], dt.float32, tag="dist")
                    for b in range(8):
                        pt = ps1.tile([128, 512], dt.float32, tag="pd")
                        for k in range(4):
                            nc.tensor.matmul(
                                pt[:],
                                lhsT=xT_t[:, k, i * 128:(i + 1) * 128],
                                rhs=embT_t[:, k, b * 512:(b + 1) * 512],
                                start=(k == 0), stop=(k == 3),
                            )
                        nc.scalar.copy(dist_t[:, b * 512:(b + 1) * 512], pt[:])
                    nc.sync.dma_start(dist_o[i * 128:(i + 1) * 128, :], dist_t[:])

                    max8 = blk.tile([128, 8], dt.float32, tag="max8")
                    idx8 = blk.tile([128, 8], dt.uint32, tag="idx8")
                    nc.vector.max(out=max8[:], in_=dist_t[:])
                    nc.vector.max_index(out=idx8[:], in_max=max8[:], in_values=dist_t[:])

                    idxi = blk.tile([128, 2], dt.int32, tag="idxi")
                    idxf = blk.tile([128, 2], dt.float32, tag="idxf")
                    nc.vector.tensor_copy(idxi[:], idx8[:, 0:2])
                    nc.vector.tensor_copy(idxf[:], idx8[:, 0:2])

                    # exact fp32 rescore of top-2 candidates
                    e1 = blk.tile([128, D], dt.float32, tag="e1")
                    e2 = blk.tile([128, D], dt.float32, tag="e2")
                    nc.gpsimd.indirect_dma_start(
                        out=e1[:], out_offset=None, in_=emb[:],
                        in_offset=bass.IndirectOffsetOnAxis(ap=idxi[:, 0:1], axis=0))
                    nc.gpsimd.indirect_dma_start(
                        out=e2[:], out_offset=None, in_=emb[:],
                        in_offset=bass.IndirectOffsetOnAxis(ap=idxi[:, 1:2], axis=0))
                    scr1 = blk.tile([128, D], dt.float32, tag="scr1")
                    scr2 = blk.tile([128, D], dt.float32, tag="scr2")
                    d1 = blk.tile([128, 1], dt.float32, tag="d1")
                    d2 = blk.tile([128, 1], dt.float32, tag="d2")
                    nc.vector.tensor_mul(scr1[:], xnat_t[:, i, :], e1[:])
                    nc.vector.tensor_mul(scr2[:], xnat_t[:, i, :], e2[:])
                    nc.vector.tensor_reduce(out=d1[:], in_=scr1[:],
                                            axis=mybir.AxisListType.X, op=Alu.add)
                    nc.vector.tensor_reduce(out=d2[:], in_=scr2[:],
                                            axis=mybir.AxisListType.X, op=Alu.add)
                    better = blk.tile([128, 1], dt.float32, tag="bet")
                    nc.vector.tensor_tensor(out=better[:], in0=d2[:], in1=d1[:],
                                            op=Alu.is_gt)
                    # corrected index = idx0 + better*(idx1-idx0)
                    di = blk.tile([128, 1], dt.float32, tag="di")
                    nc.vector.tensor_sub(di[:], idxf[:, 1:2], idxf[:, 0:1])
                    nc.vector.tensor_scalar_mul(di[:], di[:], better[:, 0:1])
                    nc.vector.tensor_add(indf_loc[:, i:i + 1], di[:], idxf[:, 0:1])
                    nc.vector.tensor_copy(indi_loc[:, i:i + 1], indf_loc[:, i:i + 1])
                    # quantize rows = e1 + better*(e2-e1)
                    q_t = blk.tile([128, D], dt.float32, tag="q")
                    nc.gpsimd.tensor_sub(q_t[:], e2[:], e1[:])
                    nc.gpsimd.tensor_scalar_mul(q_t[:], q_t[:], better[:, 0:1])
                    nc.gpsimd.tensor_add(q_t[:], q_t[:], e1[:])
                    nc.sync.dma_start(quant_o[i * 128:(i + 1) * 128, :], q_t[:])

                nc.sync.dma_start(
                    ind_o.rearrange("(j p) -> p j", p=128), indi_loc[:])

            # ================= Phase 1.5: AllGather indices =================
            if KPHASE >= 2:
                ag_in = dr.tile([NL, 1], dt.int32)
                ag_out = dr.tile([NT, 1], dt.int32, addr_space="Shared")
                nc.sync.dma_start(ag_in.rearrange("(j p) o -> p (j o)", p=128),
                                  indi_loc[:])
                nc.gpsimd.collective_compute(
                    "AllGather", Alu.bypass,
                    replica_groups=[list(range(NCORES))],
                    ins=[ag_in.opt()], outs=[ag_out.opt()])

            # ================= Phase 2: embed_sum + bins for code shard ======
            if KPHASE >= 3:
              with (
                tc.tile_pool(name="p2", bufs=4) as p2,
                tc.tile_pool(name="p3", bufs=1) as p3,
                tc.tile_pool(name="ps2", bufs=1, space="PSUM") as ps2,
              ):
                indall_i = p3.tile([128, NJ], dt.int32)
                indall_f = p3.tile([128, NJ], dt.float32)
                nc.sync.dma_start(indall_i[:],
                                  ag_out.rearrange("(j p) o -> p (j o)", p=128))
                nc.vector.tensor_copy(indall_f[:], indall_i[:])

                es_a = [ps2.tile([128, 257], dt.float32, tag=f"esa{k}",
                                 name=f"esa{k}") for k in range(CCH)]
                es_b = [ps2.tile([128, 257], dt.float32, tag=f"esb{k}",
                                 name=f"esb{k}") for k in range(CCH)]

                for j in range(NJ):
                    xa = p2.tile([128, 514], dt.bfloat16, tag="xa")
                    nc.sync.dma_start(xa[:], xaug[j * 128:(j + 1) * 128, :])
                    mask = p2.tile([128, 512], dt.bfloat16, tag="mask")
                    # mask[p, c] = ((iota[c] + cbase) == ind_global[j*128+p])
                    eng = nc.vector if j % 2 == 0 else nc.gpsimd
                    eng.tensor_scalar(
                        out=mask[:], in0=iota_t[:],
                        scalar1=cbase_t[:, 0:1], scalar2=indall_f[:, j:j + 1],
                        op0=Alu.add, op1=Alu.is_equal)
                    for k in range(CCH):
                        nc.tensor.matmul(
                            es_a[k][:], lhsT=mask[:, k * 128:(k + 1) * 128],
                            rhs=xa[:, 0:257], start=(j == 0), stop=(j == NJ - 1))
                        nc.tensor.matmul(
                            es_b[k][:], lhsT=mask[:, k * 128:(k + 1) * 128],
                            rhs=xa[:, 257:514], start=(j == 0), stop=(j == NJ - 1))

                # ============= Phase 3: EMA + l2norm on shard ============
                es_sb = p3.tile([128, CCH, D], dt.float32)
                bins = p3.tile([128, CCH], dt.float32)
                for k in range(CCH):
                    nc.scalar.copy(es_sb[:, k, 0:256], es_a[k][:, 0:256])
                    nc.scalar.copy(es_sb[:, k, 256:512], es_b[k][:, 0:256])
                    nc.scalar.copy(bins[:, k:k + 1], es_a[k][:, 256:257])

                ea_t = p3.tile([128, CCH, D], dt.float32)
                nc.sync.dma_start(ea_t[:], ea_sh.rearrange("(k p) d -> p k d", p=128))
                # nea = DECAY*ea + (1-DECAY)*es
                nc.vector.tensor_scalar(out=es_sb[:], in0=es_sb[:],
                                        scalar1=1.0 - DECAY, scalar2=None,
                                        op0=Alu.mult)
                nea = p3.tile([128, CCH, D], dt.float32)
                nc.vector.scalar_tensor_tensor(
                    out=nea[:], in0=ea_t[:], scalar=DECAY, in1=es_sb[:],
                    op0=Alu.mult, op1=Alu.add)
                nc.sync.dma_start(nea_o.rearrange("(k p) d -> p k d", p=128), nea[:])

                # ncs = DECAY*cs + (1-DECAY)*bins
                cs_t = p3.tile([128, CCH], dt.float32)
                nc.sync.dma_start(cs_t[:], cs_sh.rearrange("(k p) -> p k", p=128))
                ncs = p3.tile([128, CCH], dt.float32)
                nc.vector.tensor_scalar(out=bins[:], in0=bins[:],
                                        scalar1=1.0 - DECAY, scalar2=None,
                                        op0=Alu.mult)
                nc.vector.scalar_tensor_tensor(
                    out=ncs[:], in0=cs_t[:], scalar=DECAY, in1=bins[:],
                    op0=Alu.mult, op1=Alu.add)
                nc.sync.dma_start(ncs_o.rearrange("(k p) -> p k", p=128), ncs[:])

                # new_embed = nea / ||nea||  (laplace divisor cancels in l2norm)
                sq = p3.tile([128, CCH, D], dt.float32)
                nc.vector.tensor_mul(sq[:], nea[:], nea[:])
                ss = p3.tile([128, CCH], dt.float32)
                nc.vector.tensor_reduce(out=ss[:], in_=sq[:],
                                        axis=mybir.AxisListType.X, op=Alu.add)
                # rnorm = 1/sqrt(ss): ACT Sqrt + one Newton step + DVE reciprocal
                y0 = p3.tile([128, CCH], dt.float32)
                nc.scalar.activation(y0[:], ss[:],
                                     mybir.ActivationFunctionType.Sqrt)
                ry = p3.tile([128, CCH], dt.float32)
                nc.vector.reciprocal(ry[:], y0[:])
                # y1 = 0.5*(y0 + ss/y0)
                y1 = p3.tile([128, CCH], dt.float32)
                nc.vector.tensor_mul(y1[:], ss[:], ry[:])
                nc.vector.tensor_add(y1[:], y1[:], y0[:])
                nc.vector.tensor_scalar(out=y1[:], in0=y1[:], scalar1=0.5,
                                        scalar2=None, op0=Alu.mult)
                rn = p3.tile([128, CCH, 1], dt.float32)
                nc.vector.reciprocal(rn[:, :, 0], y1[:])
                nemb = p3.tile([128, CCH, D], dt.float32)
                nc.vector.tensor_tensor(out=nemb[:], in0=nea[:],
                                        in1=rn[:].to_broadcast([128, CCH, D]),
                                        op=Alu.mult)
                nc.sync.dma_start(nemb_o.rearrange("(k p) d -> p k d", p=128),
                                  nemb[:])

    nc.finalize()
    return nc


def kernel(x, embed, cluster_size, embed_avg):
    x = np.ascontiguousarray(np.asarray(x, dtype=np.float32))
    embed = np.ascontiguousarray(np.asarray(embed, dtype=np.float32))
    cluster_size = np.asarray(cluster_size, dtype=np.float32)
    embed_avg = np.asarray(embed_avg, dtype=np.float32)

    x2 = x.reshape(NT, D)
    e0 = embed[0]                       # [C, D]
    ea0 = embed_avg[0]                  # [C, D]
    cs0 = cluster_size[0]               # [C]

    embT = np.ascontiguousarray(e0.T)
    ones = np.ones((NT, 1), dtype=np.float32)
    xaug = np.concatenate([x2[:, :256], ones, x2[:, 256:], ones],
                          axis=1).astype(ml_dtypes.bfloat16)

    in_maps = []
    for r in range(NCORES):
        xl = x2[r * NL:(r + 1) * NL]
        in_maps.append({
            "xT": np.ascontiguousarray(xl.T),
            "xnat": np.ascontiguousarray(xl),
            "embT": embT,
            "emb": e0,
            "xaug": xaug,
            "ea_sh": np.ascontiguousarray(ea0[r * CL:(r + 1) * CL]),
            "cs_sh": np.ascontiguousarray(cs0[r * CL:(r + 1) * CL]),
            "cbase": np.full((128, 1), r * CL, dtype=np.float32),
        })

    if "nc" not in _CACHE:
        _CACHE["nc"] = _build()
    res = run_bass_kernel_spmd(_CACHE["nc"], in_maps,
                               core_ids=list(range(NCORES)))
    _CACHE["last_res"] = res
    rs = res.results

    quantize = np.concatenate([rs[r]["quant_o"] for r in range(NCORES)], axis=0)
    embed_ind = np.concatenate([rs[r]["ind_o"] for r in range(NCORES)], axis=0)
    dist = np.concatenate([rs[r]["dist_o"] for r in range(NCORES)], axis=0)
    new_embed = np.concatenate([rs[r]["nemb_o"] for r in range(NCORES)], axis=0)
    new_cs = np.concatenate([rs[r]["ncs_o"] for r in range(NCORES)], axis=0)
    new_ea = np.concatenate([rs[r]["nea_o"] for r in range(NCORES)], axis=0)

    return (
        quantize.reshape(B, N, D),
        embed_ind.reshape(B, N).astype(np.int32),
        dist.reshape(H, B, N, C),
        new_embed.reshape(H, C, D),
        new_cs.reshape(H, C),
        new_ea.reshape(H, C, D),
    )
